# revision 21
# baseline (speedup 1.0000x reference)
"""Bidirectional masked softmax geometric-mean kernel for Trainium2 (8 cores).

Problem: for each batch b (8 total):
  mask[i,j] = (i < L1_b) & (j < L2_b)
  logits    = where(mask, sim/TAU, -1e30)
  out       = where(mask, sqrt(EPS + softmax_row(logits) * softmax_col(logits)), 0)

Sharding: data-parallel over batch: core c handles slab c ([2048,2048] f32).

Math: with a fixed global stabilizer M (valid upper bound on logits),
  row_sm * col_sm = E^2 / (R_i * C_j),  E = exp(x/TAU - M),
  R_i = sum_j E (masked), C_j = sum_i E (masked)
so no per-row/col max pass is needed; exp underflow is benign because the
EPS floor dominates anything below 1e-8.

v3 layout — every phase pinned to the DMA roofline (~42us in + ~42us out,
16 queues x ~26GB/s; input must fully land before any output exists, so the
two streams can't overlap):
  startup: cmask arrives as a 4KB f32r row; a K=1 ones-matmul broadcasts it
        to PSUM and idle ACT/DVE convert to bf16 masks before exp(tile 0) —
        no 512KB broadcast DMA competing with the x stream.
  pass1 per 128-row tile: ACT exp(2x + rbias) -> E in BF16 with accum_out row
        sums; DVE right-half STT accumulates the invalid-column tail Rbad and
        a bf16 square E2 = E*E into its own buffer; PE colsum matmuls with a
        [128,1] ones stationary chain into 4 PSUM banks ([1,512] rows — a
        [128,128] ones stationary broadcasts for free but its 32KB LDWEIGHTS
        per matmul both exceeds the DMA cadence and steals SBUF bandwidth
        from the x stream; measured pass1 lost ~8us that way).
  mid:  batched invR on [128,16]; DVE copies the [1,512] colsum rows to bf16,
        K=1 ones-matmuls broadcast them into the other 4 PSUM banks, and the
        otherwise-idle ACT does invC = exp(-ln(C)) (ln and exp share the
        loaded table set), writing bf16 directly; then a dummy sqrt pulls
        the one unavoidable table swap ahead of pass 2.
  pass2 per tile: DVE bf16 mul Q = E2 * invC (2x DVE mode) -> ACT
        sqrt(Q * invR_i + EPS*rmask_i) -> f32, DVE right-half mul by col
        mask -> DMA out.
"""

import numpy as np
from contextlib import ExitStack

import concourse.bass as bass
import concourse.mybir as mybir
import concourse.tile as tile
from concourse.bass_utils import run_bass_kernel_spmd

B = 8
L = 2048
P = 128
NT = L // P  # 16 row tiles
TAU = 0.5
EPS = 1e-8
MSTAB = 24.0  # global stabilizer in logit (x/TAU) units; logits are within ~±11
NEGB = 30000.0  # additive -inf substitute (exp underflows to exactly 0)
F32 = mybir.dt.float32
F32R = mybir.dt.float32r
BF16 = mybir.dt.bfloat16

HALF = 1024  # lengths are >= 1024, so columns [0, 1024) are always valid
CH = 512  # matmul free-dim chunk (PSUM bank limit)
NCH = L // CH  # 4 colsum accumulation chains

_CACHE = {}


def _body(ctx, tc, x, cmask, auxT, y):
    nc = tc.nc
    Exp = mybir.ActivationFunctionType.Exp
    Sqrt = mybir.ActivationFunctionType.Sqrt
    Ln = mybir.ActivationFunctionType.Ln
    Copy = mybir.ActivationFunctionType.Copy
    mult = mybir.AluOpType.mult
    add = mybir.AluOpType.add

    singles = ctx.enter_context(tc.tile_pool(name="singles", bufs=1))
    xpool = ctx.enter_context(tc.tile_pool(name="xp", bufs=4))
    epool = ctx.enter_context(tc.tile_pool(name="ep", bufs=3))
    e2pool = ctx.enter_context(tc.tile_pool(name="e2p", bufs=NT))
    qpool = ctx.enter_context(tc.tile_pool(name="qp", bufs=3))
    # deep enough that sqrt(t) never waits on the out-DMA of tile t-bufs
    # riding a jittery queue
    ypool = ctx.enter_context(tc.tile_pool(name="yp", bufs=6))
    pspool = ctx.enter_context(tc.tile_pool(name="ps", bufs=NCH, space="PSUM"))
    # shared broadcast pool (2 rotating slots): the startup cmask broadcast
    # (dead by pass 1), then the 4 mid C-broadcasts — each is consumed by
    # its chunked ln right behind the matmul, so 2 slots pipeline cleanly
    bcpool = ctx.enter_context(tc.tile_pool(name="bc", bufs=2, space="PSUM"))

    # --- constants / per-row vectors ---
    # x tile 0 first: its 1MB dominates the first-exp critical path; the
    # small aux/cmask rows draft behind it on the queues
    x0 = xpool.tile([P, L], F32, tag="xt")
    nc.sync.dma_start(out=x0, in_=x[0:P, :])
    aux_sb = singles.tile([P, 4 * NT], F32, tag="aux")
    nc.sync.dma_start(out=aux_sb, in_=auxT[:, :])
    # cmask right half as a single 4KB f32r row; broadcast on-chip
    cmrow = singles.tile([1, L - HALF], F32R, tag="cmrow")
    nc.sync.dma_start(out=cmrow, in_=cmask[0:1, HALF:])

    rbias_sb = aux_sb[:, 0:NT]
    sbias_sb = aux_sb[:, NT : 2 * NT]
    rfix_sb = aux_sb[:, 2 * NT : 3 * NT]

    ones_bf = singles.tile([P, 1], BF16, tag="ones_bf")
    nc.vector.memset(ones_bf, 1.0)
    ones_col = singles.tile([1, P], F32, tag="ones_col")
    nc.vector.memset(ones_col, 1.0)
    ones_col_bf = singles.tile([1, P], BF16, tag="ones_col_bf")
    nc.vector.memset(ones_col_bf, 1.0)
    # dummy 1-wide exp: pulls the ACT_TABLE_LOAD for the exp/ln set to kernel
    # start instead of serializing it ahead of exp(tile 0)
    warm = singles.tile([P, 1], F32, tag="warm")
    nc.scalar.activation(warm, ones_bf, Exp)

    # broadcast cmask to [128, 1024] via K=1 matmul into 2 spare PSUM banks,
    # then derive both bf16 masks on idle engines before pass 1 needs them
    cmb = [bcpool.tile([P, CH], F32, tag="cmb", name=f"cmb{h}") for h in range(2)]
    for h in range(2):
        nc.tensor.matmul(
            cmb[h][:, :],
            ones_col.bitcast(F32R),
            cmrow[0:1, h * CH : (h + 1) * CH],
            start=True,
            stop=True,
        )
    cmask_h = singles.tile([P, L - HALF], BF16, tag="cmask_h")
    ncmask_h = singles.tile([P, L - HALF], BF16, tag="ncmask_h")
    for h in range(2):
        sl = slice(h * CH, (h + 1) * CH)
        nc.scalar.activation(cmask_h[:, sl], cmb[h][:, :], Copy)
        nc.vector.tensor_scalar(ncmask_h[:, sl], cmb[h][:, :], -1.0, 1.0, mult, add)

    Rsum = singles.tile([P, NT], F32, tag="Rsum")
    Rbad = singles.tile([P, NT], F32, tag="Rbad")
    invR = singles.tile([P, NT], F32, tag="invR")
    Crow_bf = singles.tile([1, L], BF16, tag="Crow_bf")
    Cln = singles.tile([P, L], F32, tag="Cln")
    invC_sb = singles.tile([P, L], BF16, tag="invC")
    sc_bad = singles.tile([P, HALF], BF16, tag="sc_bad")  # dead STT output

    # 4 colsum accumulators [1, 512], one PSUM bank each; chain over t
    Cps = [pspool.tile([1, CH], F32, tag="Cps", name=f"Cps{c}") for c in range(NCH)]

    # --- pass 1: E = exp(2x + rbias) in bf16, UNMASKED in columns (row
    #     masking via rbias). R = full rowsum (exp accum) minus the
    #     invalid-column tail (right-half STT accum; cols < 1024 are always
    #     valid since L2 >= 1024). E2 = E*E lands in its own bf16 buffer.
    #     Colsums don't need column masking: invalid columns' C values are
    #     garbage but finite, and those outputs get zeroed at the end. ---
    e2_tiles = []
    for t in range(NT):
        if t == 0:
            xt = x0
        else:
            xt = xpool.tile([P, L], F32, tag="xt")
            nc.sync.dma_start(out=xt, in_=x[t * P : (t + 1) * P, :])
        Et = epool.tile([P, L], BF16, tag="Et")
        nc.scalar.activation(
            Et,
            xt,
            Exp,
            bias=rbias_sb[:, t : t + 1],
            scale=2.0,
            accum_out=Rsum[:, t : t + 1],
        )
        # Rbad[:, t] = sum_j>=L2 E
        nc.vector.scalar_tensor_tensor(
            sc_bad,
            Et[:, HALF:],
            1.0,
            ncmask_h,
            mult,
            mult,
            accum_out=Rbad[:, t : t + 1],
        )
        E2t = e2pool.tile([P, L], BF16, tag="E2", name=f"E2_{t}")
        if t < NT - 1:
            # tile 15's square is deferred past the mid-critical DVE ops:
            # it would otherwise sit between the last STT and the Crow
            # copies, delaying the whole invC chain by its 1.2us
            nc.vector.tensor_mul(E2t, Et, Et)
        e2_tiles.append(E2t)
        last_Et = Et
        for c in range(NCH):
            nc.tensor.matmul(
                Cps[c][:, :],
                ones_bf,
                Et[:, c * CH : (c + 1) * CH],
                start=(t == 0),
                stop=(t == NT - 1),
            )

    # --- mid ---
    # invC = exp(-ln(C)) on idle ACT after a cheap 128-way broadcast: DVE
    # copies each [1,512] colsum row to bf16 as its chain stops, a K=1
    # ones-matmul broadcasts it into a spare PSUM bank, chunked ln reads
    # PSUM as each chunk lands. ln stays f32 (bf16 u would cost ~5% in
    # exp(-u)); the exp lands bf16 directly. C > 0 always. Invalid columns
    # carry garbage-but-finite invC; their outputs are zeroed at the end.
    for c in range(NCH):
        sl = slice(c * CH, (c + 1) * CH)
        nc.vector.tensor_scalar_mul(Crow_bf[0:1, sl], Cps[c][:, :], 1.0)
        Cb = bcpool.tile([P, CH], F32, tag="Cb", name=f"Cb{c}")
        nc.tensor.matmul(
            Cb[:, :], ones_col_bf, Crow_bf[0:1, sl], start=True, stop=True
        )
        nc.scalar.activation(Cln[:, sl], Cb[:, :], Ln)
    nc.scalar.activation(invC_sb, Cln, Exp, scale=-1.0)

    nc.vector.tensor_sub(Rsum, Rsum, Rbad)
    nc.vector.tensor_add(Rsum, Rsum, rfix_sb)
    nc.vector.reciprocal(invR, Rsum)
    nc.vector.tensor_mul(e2_tiles[NT - 1], last_Et, last_Et)
    # pull the sqrt ACT table swap ahead of the first real sqrt; reading
    # invC_sb (not a constant) pins it after the mid exp — the scheduler is
    # dependency-driven and would otherwise hoist it to kernel start, where
    # the swap gets undone by the first pass-1 exp
    nc.scalar.activation(warm, invC_sb[:, 0:1], Sqrt)

    # --- pass 2: out = cmask * sqrt(E2 * invC * invR_i + EPS*rmask_i) ---
    # tile 0 is split into column halves purely for latency: the left half
    # (always-valid columns) needs no mask, so its out-DMA starts one
    # half-sqrt after invC lands instead of a full TT+sqrt+mask later
    y_tiles = []

    def finish_tile(tt):
        # right-half col mask on DVE, one tile late so sqrt(tt) is done by
        # the time the mask reaches the DVE queue head
        yt = y_tiles[tt]
        nc.vector.tensor_mul(yt[:, HALF:], yt[:, HALF:], cmask_h)
        nc.sync.dma_start(out=y[tt * P : (tt + 1) * P, :], in_=yt)

    for t in range(NT):
        Qt = qpool.tile([P, L], BF16, tag="Qt")
        yt = ypool.tile([P, L], F32, tag="yt")
        if t == 0:
            nc.vector.tensor_mul(Qt[:, :HALF], e2_tiles[0][:, :HALF], invC_sb[:, :HALF])
            nc.vector.tensor_mul(Qt[:, HALF:], e2_tiles[0][:, HALF:], invC_sb[:, HALF:])
            nc.scalar.activation(
                yt[:, :HALF],
                Qt[:, :HALF],
                Sqrt,
                bias=sbias_sb[:, 0:1],
                scale=invR[:, 0:1],
            )
            nc.sync.dma_start(out=y[0:P, 0:HALF], in_=yt[:, :HALF])
            nc.scalar.activation(
                yt[:, HALF:],
                Qt[:, HALF:],
                Sqrt,
                bias=sbias_sb[:, 0:1],
                scale=invR[:, 0:1],
            )
        else:
            nc.vector.tensor_mul(Qt, e2_tiles[t], invC_sb)
            nc.scalar.activation(
                yt, Qt, Sqrt, bias=sbias_sb[:, t : t + 1], scale=invR[:, t : t + 1]
            )
        y_tiles.append(yt)
        if t == 1:
            # tile 0: only the right half remains (left already shipped)
            nc.vector.tensor_mul(
                y_tiles[0][:, HALF:], y_tiles[0][:, HALF:], cmask_h
            )
            nc.sync.dma_start(out=y[0:P, HALF:], in_=y_tiles[0][:, HALF:])
        elif t >= 2:
            finish_tile(t - 1)
    finish_tile(NT - 1)


def _split_multi_waits(nc):
    """This walrus build's CoreV3 setupSyncWait rejects ANY instruction
    carrying more than one semaphore wait ("Too many sync wait commands");
    the ISA Events header has a single wait slot. Hoist extra waits onto
    preceding same-engine NoOps (sequential ge-waits on monotonic semaphores
    are equivalent to a combined wait). Apply only for the HW path — the
    synthetic NoOps lack the sim's sem bookkeeping and break CoreSim."""
    n = 0
    for fn in nc.m.functions:
        for bb in fn.blocks:
            out = []
            changed = False
            for inst in bb.instructions:
                si = inst.sync_info
                waits = list(si.on_wait) if (si and si.on_wait) else []
                if len(waits) > 1:
                    for w in waits[:-1]:
                        n += 1
                        out.append(
                            mybir.InstNoOp(
                                name=f"antsplitwait-{n}",
                                engine=inst.engine,
                                sync_info=mybir.SyncInfo(on_wait=[w], on_update=[]),
                            )
                        )
                    si.on_wait = waits[-1:]
                    changed = True
                out.append(inst)
            if changed:
                bb.instructions = out
    return nc


def build_nc(split_waits=True):
    nc = bass.Bass()
    x = nc.dram_tensor("x", [L, L], F32, kind="ExternalInput")
    # f32r so the K=1 broadcast matmul's input writer (the DMA) is
    # trivially "f32r-rounded" for the BIR verifier; 0/1 are exact
    cmask = nc.dram_tensor("cmask", [1, L], F32R, kind="ExternalInput")
    auxT = nc.dram_tensor("auxT", [P, 4 * NT], F32, kind="ExternalInput")
    y = nc.dram_tensor("y", [L, L], F32, kind="ExternalOutput")

    with tile.TileContext(nc) as tc, ExitStack() as ctx:
        with nc.allow_low_precision("bf16 intermediates; EPS floor dominates"):
            _body(ctx, tc, x, cmask, auxT, y)
    if split_waits:
        _split_multi_waits(nc)
    return nc


def get_nc():
    if "nc" not in _CACHE:
        _CACHE["nc"] = build_nc()
    return _CACHE["nc"]


def make_in_maps(sim_matrix, lengths):
    sim_matrix = np.ascontiguousarray(np.asarray(sim_matrix, dtype=np.float32))
    lengths = np.asarray(lengths, dtype=np.int32)
    idx = np.arange(L)
    in_maps = []
    for c in range(sim_matrix.shape[0]):
        l1, l2 = int(lengths[c, 0]), int(lengths[c, 1])
        rv = idx < l1  # row valid
        cv = idx < l2  # col valid

        def tcol(vals):  # [2048] -> [128, 16] with element i at [i%128, i//128]
            return np.ascontiguousarray(
                np.asarray(vals, dtype=np.float32).reshape(NT, P).T
            )

        auxT = np.concatenate(
            [
                tcol(np.where(rv, -MSTAB, -MSTAB - NEGB)),  # rbias
                tcol(np.where(rv, EPS, 0.0)),  # sbias
                tcol(np.where(rv, 0.0, 1.0)),  # rfix
                tcol(np.where(cv, 0.0, 1.0)),  # cfix (unused)
            ],
            axis=1,
        )
        in_maps.append(
            {
                "x": sim_matrix[c],
                "cmask": cv.astype(np.float32)[None, :],
                "auxT": np.ascontiguousarray(auxT),
            }
        )
    return in_maps


def run(sim_matrix, lengths, trace=False):
    nc = get_nc()
    in_maps = make_in_maps(sim_matrix, lengths)
    res = run_bass_kernel_spmd(nc, in_maps, list(range(len(in_maps))), trace=trace)
    out = np.stack([res.results[c]["y"] for c in range(len(in_maps))], axis=0)
    return out, res


def kernel(sim_matrix, lengths):
    out, _ = run(sim_matrix, lengths, trace=False)
    return out


# revision 24
# speedup vs baseline: 1.1637x; 1.1637x over previous
"""Bidirectional masked softmax geometric-mean kernel for Trainium2 (8 cores).

Problem: for each batch b (8 total):
  mask[i,j] = (i < L1_b) & (j < L2_b)
  logits    = where(mask, sim/TAU, -1e30)
  out       = where(mask, sqrt(EPS + softmax_row(logits) * softmax_col(logits)), 0)

Sharding: data-parallel over batch: core c handles slab c ([2048,2048] f32).

Math: with a fixed global stabilizer M (valid upper bound on logits),
  row_sm * col_sm = E^2 / (R_i * C_j),  E = exp(x/TAU - M),
  R_i = sum_j E (masked), C_j = sum_i E (masked)
so no per-row/col max pass is needed; exp underflow is benign because the
EPS floor dominates anything below 1e-8.

v3 layout — every phase pinned to the DMA roofline (~42us in + ~42us out,
16 queues x ~26GB/s; input must fully land before any output exists, so the
two streams can't overlap):
  startup: cmask arrives as a 4KB f32r row; a K=1 ones-matmul broadcasts it
        to PSUM and idle ACT/DVE convert to bf16 masks before exp(tile 0) —
        no 512KB broadcast DMA competing with the x stream.
  pass1 per 128-row tile: ACT exp(2x + rbias) -> E in BF16 with accum_out row
        sums; DVE right-half STT accumulates the invalid-column tail Rbad and
        a bf16 square E2 = E*E into its own buffer; PE colsum matmuls with a
        [128,1] ones stationary chain into 4 PSUM banks ([1,512] rows — a
        [128,128] ones stationary broadcasts for free but its 32KB LDWEIGHTS
        per matmul both exceeds the DMA cadence and steals SBUF bandwidth
        from the x stream; measured pass1 lost ~8us that way).
  mid:  batched invR on [128,16]; DVE copies the [1,512] colsum rows to bf16,
        K=1 ones-matmuls broadcast them into the other 4 PSUM banks, and the
        otherwise-idle ACT does invC = exp(-ln(C)) (ln and exp share the
        loaded table set), writing bf16 directly; then a dummy sqrt pulls
        the one unavoidable table swap ahead of pass 2.
  pass2 per tile: DVE bf16 mul Q = E2 * invC (2x DVE mode) -> ACT
        sqrt(Q * invR_i + EPS*rmask_i) -> f32, DVE right-half mul by col
        mask -> DMA out.
"""

import numpy as np
from contextlib import ExitStack

import concourse.bass as bass
import concourse.mybir as mybir
import concourse.tile as tile
from concourse.bass_utils import run_bass_kernel_spmd

B = 8
L = 2048
P = 128
NT = L // P  # 16 row tiles
TAU = 0.5
EPS = 1e-8
MSTAB = 24.0  # global stabilizer in logit (x/TAU) units; logits are within ~±11
NEGB = 30000.0  # additive -inf substitute (exp underflows to exactly 0)
F32 = mybir.dt.float32
F32R = mybir.dt.float32r
BF16 = mybir.dt.bfloat16

HALF = 1024  # lengths are >= 1024, so columns [0, 1024) are always valid
CH = 512  # matmul free-dim chunk (PSUM bank limit)
NCH = L // CH  # 4 colsum accumulation chains

_CACHE = {}


def _body(ctx, tc, x, cmask, auxT, y):
    nc = tc.nc
    Exp = mybir.ActivationFunctionType.Exp
    Sqrt = mybir.ActivationFunctionType.Sqrt
    Ln = mybir.ActivationFunctionType.Ln
    Copy = mybir.ActivationFunctionType.Copy
    mult = mybir.AluOpType.mult
    add = mybir.AluOpType.add

    singles = ctx.enter_context(tc.tile_pool(name="singles", bufs=1))
    xpool = ctx.enter_context(tc.tile_pool(name="xp", bufs=4))
    epool = ctx.enter_context(tc.tile_pool(name="ep", bufs=3))
    e2pool = ctx.enter_context(tc.tile_pool(name="e2p", bufs=NT))
    qpool = ctx.enter_context(tc.tile_pool(name="qp", bufs=3))
    # deep enough that sqrt(t) never waits on the out-DMA of tile t-bufs
    # riding a jittery queue
    ypool = ctx.enter_context(tc.tile_pool(name="yp", bufs=6))
    pspool = ctx.enter_context(tc.tile_pool(name="ps", bufs=NCH, space="PSUM"))
    # shared broadcast pool (2 rotating slots): the startup cmask broadcast
    # (dead by pass 1), then the 4 mid C-broadcasts — each is consumed by
    # its chunked ln right behind the matmul, so 2 slots pipeline cleanly
    bcpool = ctx.enter_context(tc.tile_pool(name="bc", bufs=2, space="PSUM"))

    # --- constants / per-row vectors ---
    # x tile 0 first: its 1MB dominates the first-exp critical path; the
    # small aux/cmask rows draft behind it on the queues
    x0 = xpool.tile([P, L], F32, tag="xt")
    nc.sync.dma_start(out=x0, in_=x[0:P, :])
    aux_sb = singles.tile([P, 4 * NT], F32, tag="aux")
    nc.sync.dma_start(out=aux_sb, in_=auxT[:, :])
    # cmask right half as a single 4KB f32r row; broadcast on-chip
    cmrow = singles.tile([1, L - HALF], F32R, tag="cmrow")
    nc.sync.dma_start(out=cmrow, in_=cmask[0:1, HALF:])

    rbias_sb = aux_sb[:, 0:NT]
    sbias_sb = aux_sb[:, NT : 2 * NT]
    rfix_sb = aux_sb[:, 2 * NT : 3 * NT]

    ones_bf = singles.tile([P, 1], BF16, tag="ones_bf")
    nc.vector.memset(ones_bf, 1.0)
    ones_col = singles.tile([1, P], F32, tag="ones_col")
    nc.vector.memset(ones_col, 1.0)
    ones_col_bf = singles.tile([1, P], BF16, tag="ones_col_bf")
    nc.vector.memset(ones_col_bf, 1.0)
    # dummy 1-wide exp: pulls the ACT_TABLE_LOAD for the exp/ln set to kernel
    # start instead of serializing it ahead of exp(tile 0)
    warm = singles.tile([P, 1], F32, tag="warm")
    nc.scalar.activation(warm, ones_bf, Exp)

    # broadcast cmask to [128, 1024] via K=1 matmul into 2 spare PSUM banks,
    # then derive both bf16 masks on idle engines before pass 1 needs them
    cmb = [bcpool.tile([P, CH], F32, tag="cmb", name=f"cmb{h}") for h in range(2)]
    for h in range(2):
        nc.tensor.matmul(
            cmb[h][:, :],
            ones_col.bitcast(F32R),
            cmrow[0:1, h * CH : (h + 1) * CH],
            start=True,
            stop=True,
        )
    cmask_h = singles.tile([P, L - HALF], BF16, tag="cmask_h")
    ncmask_h = singles.tile([P, L - HALF], BF16, tag="ncmask_h")
    for h in range(2):
        sl = slice(h * CH, (h + 1) * CH)
        nc.scalar.activation(cmask_h[:, sl], cmb[h][:, :], Copy)
        nc.vector.tensor_scalar(ncmask_h[:, sl], cmb[h][:, :], -1.0, 1.0, mult, add)

    Rsum = singles.tile([P, NT], F32, tag="Rsum")
    Rbad = singles.tile([P, NT], F32, tag="Rbad")
    invR = singles.tile([P, NT], F32, tag="invR")
    Rext = singles.tile([P, 1], F32, tag="Rext")  # tile-15 right-half rowsum
    Crow_bf = singles.tile([1, L], BF16, tag="Crow_bf")
    Cln = singles.tile([P, L], F32, tag="Cln")
    invC_sb = singles.tile([P, L], BF16, tag="invC")
    sc_bad = singles.tile([P, HALF], BF16, tag="sc_bad")  # dead STT output

    # 4 colsum accumulators [1, 512], one PSUM bank each; chain over t
    Cps = [pspool.tile([1, CH], F32, tag="Cps", name=f"Cps{c}") for c in range(NCH)]

    # --- pass 1: E = exp(2x + rbias) in bf16, UNMASKED in columns (row
    #     masking via rbias). R = full rowsum (exp accum) minus the
    #     invalid-column tail (right-half STT accum; cols < 1024 are always
    #     valid since L2 >= 1024). E2 = E*E lands in its own bf16 buffer.
    #     Colsums don't need column masking: invalid columns' C values are
    #     garbage but finite, and those outputs get zeroed at the end. ---
    e2_tiles = []
    for t in range(NT):
        if t == 0:
            xt = x0
        else:
            xt = xpool.tile([P, L], F32, tag="xt")
            nc.sync.dma_start(out=xt, in_=x[t * P : (t + 1) * P, :])
        E2t = e2pool.tile([P, L], BF16, tag="E2", name=f"E2_{t}")
        if t < NT - 1:
            Et = epool.tile([P, L], BF16, tag="Et")
            nc.scalar.activation(
                Et,
                xt,
                Exp,
                bias=rbias_sb[:, t : t + 1],
                scale=2.0,
                accum_out=Rsum[:, t : t + 1],
            )
            # Rbad[:, t] = sum_j>=L2 E
            nc.vector.scalar_tensor_tensor(
                sc_bad,
                Et[:, HALF:],
                1.0,
                ncmask_h,
                mult,
                mult,
                accum_out=Rbad[:, t : t + 1],
            )
            nc.vector.tensor_mul(E2t, Et, Et)
            for c in range(NCH):
                nc.tensor.matmul(
                    Cps[c][:, :],
                    ones_bf,
                    Et[:, c * CH : (c + 1) * CH],
                    start=(t == 0),
                    stop=False,
                )
        else:
            # tile 15 is split into column halves (two independent tiles so
            # the deps stay half-granular): colsum chunks 0-1 run in the
            # shadow of the right half's exp, pulling every chain stop — and
            # with it the whole invC chain — ~1us earlier
            EtL = epool.tile([P, HALF], BF16, tag="EtL")
            EtR = epool.tile([P, HALF], BF16, tag="EtR")
            nc.scalar.activation(
                EtL,
                xt[:, :HALF],
                Exp,
                bias=rbias_sb[:, t : t + 1],
                scale=2.0,
                accum_out=Rsum[:, t : t + 1],
            )
            nc.scalar.activation(
                EtR,
                xt[:, HALF:],
                Exp,
                bias=rbias_sb[:, t : t + 1],
                scale=2.0,
                accum_out=Rext[:, 0:1],
            )
            for c in range(NCH):
                src = EtL if c < NCH // 2 else EtR
                off = 0 if c < NCH // 2 else HALF
                nc.tensor.matmul(
                    Cps[c][:, :],
                    ones_bf,
                    src[:, c * CH - off : (c + 1) * CH - off],
                    start=False,
                    stop=True,
                )
        e2_tiles.append(E2t)

    # --- mid ---
    # invC = exp(-ln(C)) on idle ACT after a cheap 128-way broadcast: DVE
    # copies each [1,512] colsum row to bf16 as its chain stops, a K=1
    # ones-matmul broadcasts it into a spare PSUM bank, chunked ln reads
    # PSUM as each chunk lands. ln stays f32 (bf16 u would cost ~5% in
    # exp(-u)); the exp lands bf16 directly. C > 0 always. Invalid columns
    # carry garbage-but-finite invC; their outputs are zeroed at the end.
    # DVE order matters: the copies go first (gated only by chain stops);
    # tile 15's STT/square and the invR chain have slack until the first
    # sqrt and run behind them.
    for c in range(NCH):
        sl = slice(c * CH, (c + 1) * CH)
        nc.vector.tensor_scalar_mul(Crow_bf[0:1, sl], Cps[c][:, :], 1.0)
        Cb = bcpool.tile([P, CH], F32, tag="Cb", name=f"Cb{c}")
        nc.tensor.matmul(
            Cb[:, :], ones_col_bf, Crow_bf[0:1, sl], start=True, stop=True
        )
        nc.scalar.activation(Cln[:, sl], Cb[:, :], Ln)
    nc.scalar.activation(invC_sb, Cln, Exp, scale=-1.0)

    # tile 15's deferred tail: Rbad, rowsum fixup, batched invR, square
    nc.vector.scalar_tensor_tensor(
        sc_bad,
        EtR,
        1.0,
        ncmask_h,
        mult,
        mult,
        accum_out=Rbad[:, NT - 1 : NT],
    )
    nc.vector.tensor_add(Rsum[:, NT - 1 : NT], Rsum[:, NT - 1 : NT], Rext)
    nc.vector.tensor_sub(Rsum, Rsum, Rbad)
    nc.vector.tensor_add(Rsum, Rsum, rfix_sb)
    nc.vector.reciprocal(invR, Rsum)
    nc.vector.tensor_mul(e2_tiles[NT - 1][:, :HALF], EtL, EtL)
    nc.vector.tensor_mul(e2_tiles[NT - 1][:, HALF:], EtR, EtR)
    # pull the sqrt ACT table swap ahead of the first real sqrt; reading
    # invC_sb (not a constant) pins it after the mid exp — the scheduler is
    # dependency-driven and would otherwise hoist it to kernel start, where
    # the swap gets undone by the first pass-1 exp
    nc.scalar.activation(warm, invC_sb[:, 0:1], Sqrt)

    # --- pass 2: out = cmask * sqrt(E2 * invC * invR_i + EPS*rmask_i) ---
    # tile 0 is split into column halves purely for latency: the left half
    # (always-valid columns) needs no mask, so its out-DMA starts one
    # half-sqrt after invC lands instead of a full TT+sqrt+mask later
    y_tiles = []

    def finish_tile(tt):
        # right-half col mask on DVE, one tile late so sqrt(tt) is done by
        # the time the mask reaches the DVE queue head
        yt = y_tiles[tt]
        nc.vector.tensor_mul(yt[:, HALF:], yt[:, HALF:], cmask_h)
        nc.sync.dma_start(out=y[tt * P : (tt + 1) * P, :], in_=yt)

    for t in range(NT):
        Qt = qpool.tile([P, L], BF16, tag="Qt")
        yt = ypool.tile([P, L], F32, tag="yt")
        if t == 0:
            # left half in two quarters: the first out-DMA issues after a
            # 0.7us quarter-sqrt instead of a 1.25us half-sqrt, and the
            # always-valid left columns need no mask at all
            for q in range(2):
                sq = slice(q * CH, (q + 1) * CH)
                nc.vector.tensor_mul(Qt[:, sq], e2_tiles[0][:, sq], invC_sb[:, sq])
                nc.scalar.activation(
                    yt[:, sq],
                    Qt[:, sq],
                    Sqrt,
                    bias=sbias_sb[:, 0:1],
                    scale=invR[:, 0:1],
                )
                nc.sync.dma_start(out=y[0:P, sq], in_=yt[:, sq])
            nc.vector.tensor_mul(Qt[:, HALF:], e2_tiles[0][:, HALF:], invC_sb[:, HALF:])
            nc.scalar.activation(
                yt[:, HALF:],
                Qt[:, HALF:],
                Sqrt,
                bias=sbias_sb[:, 0:1],
                scale=invR[:, 0:1],
            )
        else:
            nc.vector.tensor_mul(Qt, e2_tiles[t], invC_sb)
            nc.scalar.activation(
                yt, Qt, Sqrt, bias=sbias_sb[:, t : t + 1], scale=invR[:, t : t + 1]
            )
        y_tiles.append(yt)
        if t == 1:
            # tile 0: only the right half remains (left already shipped)
            nc.vector.tensor_mul(
                y_tiles[0][:, HALF:], y_tiles[0][:, HALF:], cmask_h
            )
            nc.sync.dma_start(out=y[0:P, HALF:], in_=y_tiles[0][:, HALF:])
        elif t >= 2:
            finish_tile(t - 1)
    finish_tile(NT - 1)


def _split_multi_waits(nc):
    """This walrus build's CoreV3 setupSyncWait rejects ANY instruction
    carrying more than one semaphore wait ("Too many sync wait commands");
    the ISA Events header has a single wait slot. Hoist extra waits onto
    preceding same-engine NoOps (sequential ge-waits on monotonic semaphores
    are equivalent to a combined wait). Apply only for the HW path — the
    synthetic NoOps lack the sim's sem bookkeeping and break CoreSim."""
    n = 0
    for fn in nc.m.functions:
        for bb in fn.blocks:
            out = []
            changed = False
            for inst in bb.instructions:
                si = inst.sync_info
                waits = list(si.on_wait) if (si and si.on_wait) else []
                if len(waits) > 1:
                    for w in waits[:-1]:
                        n += 1
                        out.append(
                            mybir.InstNoOp(
                                name=f"antsplitwait-{n}",
                                engine=inst.engine,
                                sync_info=mybir.SyncInfo(on_wait=[w], on_update=[]),
                            )
                        )
                    si.on_wait = waits[-1:]
                    changed = True
                out.append(inst)
            if changed:
                bb.instructions = out
    return nc


def build_nc(split_waits=True):
    nc = bass.Bass()
    x = nc.dram_tensor("x", [L, L], F32, kind="ExternalInput")
    # f32r so the K=1 broadcast matmul's input writer (the DMA) is
    # trivially "f32r-rounded" for the BIR verifier; 0/1 are exact
    cmask = nc.dram_tensor("cmask", [1, L], F32R, kind="ExternalInput")
    auxT = nc.dram_tensor("auxT", [P, 4 * NT], F32, kind="ExternalInput")
    y = nc.dram_tensor("y", [L, L], F32, kind="ExternalOutput")

    with tile.TileContext(nc) as tc, ExitStack() as ctx:
        with nc.allow_low_precision("bf16 intermediates; EPS floor dominates"):
            _body(ctx, tc, x, cmask, auxT, y)
    if split_waits:
        _split_multi_waits(nc)
    return nc


def get_nc():
    if "nc" not in _CACHE:
        _CACHE["nc"] = build_nc()
    return _CACHE["nc"]


def make_in_maps(sim_matrix, lengths):
    sim_matrix = np.ascontiguousarray(np.asarray(sim_matrix, dtype=np.float32))
    lengths = np.asarray(lengths, dtype=np.int32)
    idx = np.arange(L)
    in_maps = []
    for c in range(sim_matrix.shape[0]):
        l1, l2 = int(lengths[c, 0]), int(lengths[c, 1])
        rv = idx < l1  # row valid
        cv = idx < l2  # col valid

        def tcol(vals):  # [2048] -> [128, 16] with element i at [i%128, i//128]
            return np.ascontiguousarray(
                np.asarray(vals, dtype=np.float32).reshape(NT, P).T
            )

        auxT = np.concatenate(
            [
                tcol(np.where(rv, -MSTAB, -MSTAB - NEGB)),  # rbias
                tcol(np.where(rv, EPS, 0.0)),  # sbias
                tcol(np.where(rv, 0.0, 1.0)),  # rfix
                tcol(np.where(cv, 0.0, 1.0)),  # cfix (unused)
            ],
            axis=1,
        )
        in_maps.append(
            {
                "x": sim_matrix[c],
                "cmask": cv.astype(np.float32)[None, :],
                "auxT": np.ascontiguousarray(auxT),
            }
        )
    return in_maps


def run(sim_matrix, lengths, trace=False):
    nc = get_nc()
    in_maps = make_in_maps(sim_matrix, lengths)
    res = run_bass_kernel_spmd(nc, in_maps, list(range(len(in_maps))), trace=trace)
    out = np.stack([res.results[c]["y"] for c in range(len(in_maps))], axis=0)
    return out, res


def kernel(sim_matrix, lengths):
    out, _ = run(sim_matrix, lengths, trace=False)
    return out
